# revision 24
# baseline (speedup 1.0000x reference)
"""Multi-head attention (quirky Dense(d_k) variant) on 8 trn2 NeuronCores.

Sharding: data-parallel over batch (B=2), tensor-parallel over heads
(8 heads -> 4 groups of 2 heads). Core c: batch c//4, head-group c%4.
Each core is fully independent (no collectives); host sums the 4 partial
outputs per batch (Wo row-sharded -> partial sums).

fp16 datapath (PSUM accumulation stays fp32): matmuls run at 1 cycle/row
vs ~3x that for fp32-HIGH, FWL weight loads engage, and DMA bytes halve.
Scalar engine (ACT) is reserved exclusively for the softmax exp stream --
it is the pacing engine (~8.4M exp elements/core). All other elementwise
work (bias adds, PSUM evacuation, normalization) runs on the DVE.

Layouts per core (L=2048, d_model=1024, 2 heads of 16 dims):
  qT/kT pad48: head0 dims at partitions 0-15, head1 at 32-47 (32-aligned
  bases let the two heads' score matmuls target different PE row strips
  and run concurrently). v unpadded [32, L], PE-transposed to natural
  [keys, dims] with a ones column per k-tile for the softmax rowsum.

Attention runs in 4 q-phases of 512 columns; per (phase, k-tile) one
[128, 1024] score PSUM holds h0|h1 side by side so a single ACT exp
instruction covers both heads. AV accumulates h0 into av0[0:17] and h1
into av1[32:49] (separate PSUM tiles, different PE column strips ->
concurrent; separate banks so interleaved accumulation groups never
share a bank's has_written bits). Row 16/48 of the accumulators is the
softmax denominator via the ones column. Normalization: rowsum ->
DRAM roundtrip broadcast -> reciprocal_approx_fast -> DVE multiply into
oT fp16. Output projection y = [oT;1;0;oT']^T @ [Wo;bo;0;Wo'] per
128-query chunk, interleaved into the next attention phase.
"""

import math
import sys

sys.path.insert(0, "/opt/trn_rl_repo")

import numpy as np

import concourse.bass as bass
import concourse.mybir as mybir
import concourse.tile as tile
from concourse import bacc
from concourse.bass_utils import run_bass_kernel_spmd

H = 8
D_MODEL = 1024
D_K = 128          # projection width (d_model / h)
HD = 16            # per-head dim after reshape
B, L = 2, 2048
DSL = 32           # per-core slice of D_K (2 heads x 16)
DP = 48            # padded: head0 dims at 0-15, head1 at 32-47
CC = 8             # contraction chunks of 128 over d_model
NLB = 4            # k/v produced in 4 L-blocks of 512
LB = L // NLB
QP = 512           # attention q-phase width
NQP = L // QP
SCALE = 1.0 / math.sqrt(float(D_K))   # reference scales by sqrt(d_k)=sqrt(128)
F32 = mybir.dt.float32
F16 = mybir.dt.float16

_CACHE = {}


def _build_nc():
    nc = bacc.Bacc(None, target_bir_lowering=False)

    xq = nc.declare_dram_parameter("xq_t", [D_MODEL, L], F16, isOutput=False)
    xk = nc.declare_dram_parameter("xk_t", [D_MODEL, L], F16, isOutput=False)
    xv = nc.declare_dram_parameter("xv_t", [D_MODEL, L], F16, isOutput=False)
    wq = nc.declare_dram_parameter("wq", [D_MODEL, DP], F16, isOutput=False)
    wk = nc.declare_dram_parameter("wk", [D_MODEL, DP], F16, isOutput=False)
    wv = nc.declare_dram_parameter("wv", [D_MODEL, DSL], F16, isOutput=False)
    bqkv = nc.declare_dram_parameter("bqkv", [DP, 3], F32, isOutput=False)
    # rows 0-15: Wo head0; 16: bo (or 0); 17-31: zero; 32-47: Wo head1
    wo = nc.declare_dram_parameter("wo", [DP, D_MODEL], F16, isOutput=False)
    identp = nc.declare_dram_parameter("identp", [32, 32], F16, isOutput=False)
    # row 0 = ones (bias row of oT), rows 1-15 = zeros (engine memsets need
    # 32-aligned partition bases, so these rows come in via DMA instead)
    orows = nc.declare_dram_parameter("orows", [16, L], F16, isOutput=False)
    y = nc.declare_dram_parameter("y", [L, D_MODEL], F16, isOutput=True)

    import os
    dbg = os.environ.get("KERNEL_DEBUG", "0") == "1"
    if dbg:
        qt_d = nc.declare_dram_parameter("qt_d", [DP, L], F16, isOutput=True)
        kt_d = nc.declare_dram_parameter("kt_d", [DP, L], F16, isOutput=True)
        vsb_d = nc.declare_dram_parameter("vsb_d", [128, 4 * 144], F16, isOutput=True)
        et_d = nc.declare_dram_parameter("et_d", [128, 2 * QP], F16, isOutput=True)
        rbr_d = nc.declare_dram_parameter("rbr_d", [DP, QP], F32, isOutput=True)
        ot_d = nc.declare_dram_parameter("ot_d", [DP, L], F16, isOutput=True)

    Exp = mybir.ActivationFunctionType.Exp

    with tile.TileContext(nc) as tc:
        with (
            tc.tile_pool(name="const", bufs=1) as constp,
            tc.tile_pool(name="qk", bufs=1) as qkpool,
            tc.tile_pool(name="ps", bufs=1, space="PSUM") as psA,
            tc.tile_pool(name="ep", bufs=1) as epool,
            tc.tile_pool(name="yp", bufs=1) as ypool,
            tc.tile_pool(name="misc", bufs=1) as misc,
            tc.tile_pool(name="dr", bufs=1, space="DRAM") as drpool,
        ):
            scratch = drpool.tile([2 * NQP, QP], F16)
            # ---- constants (gpsimd DMA queue: needed first, tiny) ----
            wq_sb = constp.tile([128, CC, DP], F16)
            nc.gpsimd.dma_start(out=wq_sb, in_=wq[:].rearrange("(c p) d -> p c d", p=128))
            wk_sb = constp.tile([128, CC, DP], F16)
            nc.gpsimd.dma_start(out=wk_sb, in_=wk[:].rearrange("(c p) d -> p c d", p=128))
            wv_sb = constp.tile([128, CC, DSL], F16)
            nc.gpsimd.dma_start(out=wv_sb, in_=wv[:].rearrange("(c p) d -> p c d", p=128))
            wo_sb = constp.tile([DP, D_MODEL], F16)
            nc.gpsimd.dma_start(out=wo_sb, in_=wo[:])
            bias_sb = constp.tile([DP, 3], F32)
            nc.gpsimd.dma_start(out=bias_sb, in_=bqkv[:])
            ident = constp.tile([32, 32], F16)
            nc.gpsimd.dma_start(out=ident, in_=identp[:])

            qT = qkpool.tile([DP, L], F16)
            # oT rows: 0-15 h0 o^T, 16 ones (bias row), 17-31 zero, 32-47 h1
            oT = qkpool.tile([DP, L], F16)
            nc.gpsimd.dma_start(out=oT[16:32, :], in_=orows[:])

            ktb = [qkpool.tile([DP, LB], F16, name=f"ktb{_i}") for _i in range(NLB)]
            vtb = [qkpool.tile([DSL, LB], F16, name=f"vtb{_i}") for _i in range(NLB)]
            # v natural + ones cols, per k-tile t within block (36 cols):
            # [16 v_h0 | 1 | pad | 16 v_h1 | 1 | pad]
            vsb = [qkpool.tile([128, 4 * 36], F16, name=f"vsb{_i}") for _i in range(NLB)]
            for lb in range(NLB):
                v3 = vsb[lb].rearrange("p (t s) -> p t s", s=36)
                nc.gpsimd.memset(v3[:, :, 16:17], 1.0)
                nc.gpsimd.memset(v3[:, :, 34:35], 1.0)

            # ---- inputs fully resident: 8 dmodel-chunks x [128, L] each; DMA
            # in L-halves so per-partition lines are 2KB (full DMA BW) ----
            xq_sb = [qkpool.tile([128, L], F16, name=f"xq{_c}") for _c in range(CC)]
            xk_sb = [qkpool.tile([128, L], F16, name=f"xk{_c}") for _c in range(CC)]
            xv_sb = [qkpool.tile([128, L], F16, name=f"xv{_c}") for _c in range(CC)]

            def load_quarter(x_dram, sb, qtr, engine):
                c0 = qtr * QP
                for cc in range(CC):
                    engine.dma_start(
                        out=sb[cc][:, c0:c0 + QP],
                        in_=x_dram[cc * 128:(cc + 1) * 128, c0:c0 + QP],
                    )

            # ---- k or v projection for one 512-wide L-block ----
            def project_k(lb):
                l0 = lb * LB
                pb = psA.tile([DP, LB], F32, tag="ps", bufs=2, name=f"pk{lb}")
                for cc in range(CC):
                    nc.tensor.matmul(
                        pb,
                        lhsT=wk_sb[:, cc, :],
                        rhs=xk_sb[cc][:, l0:l0 + LB],
                        start=(cc == 0),
                        stop=(cc == CC - 1),
                    )
                nc.vector.tensor_scalar_add(
                    ktb[lb][0:DP, :], pb, bias_sb[0:DP, 1:2]
                )

            def project_v(lb):
                l0 = lb * LB
                pb = psA.tile([DSL, LB], F32, tag="ps", bufs=2, name=f"pv{lb}")
                for cc in range(CC):
                    nc.tensor.matmul(
                        pb,
                        lhsT=wv_sb[:, cc, 0:DSL],
                        rhs=xv_sb[cc][:, l0:l0 + LB],
                        start=(cc == 0),
                        stop=(cc == CC - 1),
                    )
                nc.vector.tensor_scalar_add(
                    vtb[lb][0:DSL, :], pb, bias_sb[0:DSL, 2:3]
                )
                # transpose this block's v into natural layout
                for i in range(4):
                    pvt = psA.tile([128, DSL], F16, tag="ps", bufs=2, name=f"pvt{lb}")
                    nc.tensor.transpose(
                        pvt, vtb[lb][:, i * 128:(i + 1) * 128], ident
                    )
                    base = i * 36
                    nc.vector.tensor_copy(vsb[lb][:, base:base + 16], pvt[:, 0:16])
                    nc.vector.tensor_copy(vsb[lb][:, base + 18:base + 34], pvt[:, 16:32])

            def qproj(c):
                psq = psA.tile([DP, QP], F32, tag="ps", bufs=2, name="psq")
                for cc in range(CC):
                    nc.tensor.matmul(
                        psq,
                        lhsT=wq_sb[:, cc, :],
                        rhs=xq_sb[cc][:, c * QP:(c + 1) * QP],
                        start=(cc == 0),
                        stop=(cc == CC - 1),
                    )
                nc.vector.tensor_scalar_add(
                    qT[:, c * QP:(c + 1) * QP], psq, bias_sb[:, 0:1]
                )

            # ---- attention phase: 512 q-cols across all 16 k-tiles ----
            def attention_phase(p, fillers):
                q0 = p * QP
                av0 = psA.tile([17, QP], F32, tag="av0", name="av0")
                av1 = psA.tile([49, QP], F32, tag="av1", name="av1")
                for t in range(16):
                    lb, ti = t // 4, t % 4
                    ps_s = psA.tile([128, 2 * QP], F32, tag="s", bufs=2, name="ps_s")
                    for h in (0, 1):
                        nc.tensor.matmul(
                            ps_s[:, h * QP:(h + 1) * QP],
                            lhsT=ktb[lb][32 * h:32 * h + HD, ti * 128:(ti + 1) * 128],
                            rhs=qT[32 * h:32 * h + HD, q0:q0 + QP],
                            start=True,
                            stop=True,
                        )
                    et = epool.tile([128, 2 * QP], F16, tag="e", bufs=2, name="et")
                    nc.scalar.activation(et, ps_s, Exp, scale=SCALE)
                    if dbg and p == 0 and t == 0:
                        nc.gpsimd.dma_start(out=et_d[:], in_=et)
                    for h, av in ((0, av0), (1, av1)):
                        nc.tensor.matmul(
                            av[32 * h:32 * h + HD + 1, :],
                            lhsT=vsb[lb][:, ti * 36 + 18 * h:ti * 36 + 18 * h + 17],
                            rhs=et[:, h * QP:(h + 1) * QP],
                            start=(t == 0),
                            stop=(t == 15),
                        )
                    if t in fillers:
                        fillers[t]()
                return av0, av1

            # ---- softmax denominator + oT normalization for one phase ----
            # Copy the raw accumulators (incl. rowsum rows 16/48) to SBUF
            # immediately so the AV PSUM banks free up for the next phase;
            # the DRAM-roundtrip rowsum broadcast then runs off the critical
            # path. Engine partition bases must be 32-aligned throughout.
            def normalize(p, av0, av1):
                q0 = p * QP
                oTu = misc.tile([49, QP], F16, tag="otu", bufs=2, name="oTu")
                nc.vector.tensor_copy(oTu[0:17, :], av0[0:17, :])
                nc.vector.tensor_copy(oTu[32:49, :], av1[32:49, :])
                nc.gpsimd.dma_start(out=scratch[2 * p:2 * p + 1, :], in_=oTu[16:17, :])
                nc.gpsimd.dma_start(
                    out=scratch[2 * p + 1:2 * p + 2, :], in_=oTu[48:49, :]
                )
                # custom DVE ops mishandle nonzero partition bases, so fill
                # rows 0-31 with h0's rowsum (16-31 unread) and run a single
                # base-0 reciprocal across all 48 rows
                rb16 = misc.tile([DP, QP], F16, tag="rb16", bufs=2, name="rb16")
                nc.gpsimd.dma_start(
                    out=rb16[0:32, :],
                    in_=scratch[2 * p:2 * p + 1, :].to_broadcast((32, QP)),
                )
                nc.gpsimd.dma_start(
                    out=rb16[32:48, :],
                    in_=scratch[2 * p + 1:2 * p + 2, :].to_broadcast((16, QP)),
                )
                rbf = misc.tile([DP, QP], F32, tag="rbf", bufs=2, name="rbf")
                nc.vector.tensor_copy(rbf[0:48, :], rb16[0:48, :])
                rbr = misc.tile([DP, QP], F32, tag="rbr", bufs=2, name="rbr")
                nc.vector.reciprocal_approx_fast(rbr[0:48, :], rbf[0:48, :])
                nc.vector.tensor_mul(
                    oT[0:16, q0:q0 + QP], oTu[0:16, :], rbr[0:16, :]
                )
                nc.vector.tensor_mul(
                    oT[32:48, q0:q0 + QP], oTu[32:48, :], rbr[32:48, :]
                )
                if dbg and p == 0:
                    nc.gpsimd.dma_start(out=rbr_d[:], in_=rbr)

            # ---- output projection for one 128-query chunk ----
            def outproj_chunk(i, evac_scalar=False):
                q0 = i * 128
                for sub in (0, 1):
                    py = psA.tile([128, 512], F32, tag="ps", bufs=2, name="py")
                    nc.tensor.matmul(
                        py,
                        lhsT=oT[:, q0:q0 + 128],
                        rhs=wo_sb[:, sub * 512:(sub + 1) * 512],
                        start=True,
                        stop=True,
                    )
                    yt = ypool.tile([128, 512], F16, tag="y", bufs=3, name="yt")
                    if evac_scalar and sub == 1:
                        nc.scalar.copy(yt, py)
                    else:
                        nc.vector.tensor_copy(yt, py)
                    nc.sync.dma_start(
                        out=y[q0:q0 + 128, sub * 512:(sub + 1) * 512], in_=yt
                    )

            # ---- schedule ----
            # parallel DMA queues, quarter-granular so phase 0 starts early:
            # sync=k, vector=v, gpsimd=weights then q
            for qtr in range(NQP):
                load_quarter(xk, xk_sb, qtr, nc.sync)
                load_quarter(xv, xv_sb, qtr, nc.scalar)
                load_quarter(xq, xq_sb, qtr, nc.gpsimd)

            project_k(0)
            qproj(0)
            project_v(0)
            project_k(1)
            project_v(1)

            avs = {}
            # phase 0: project k/v blocks 2-3 and q chunk 1
            avs[0] = attention_phase(0, {
                2: lambda: project_k(2),
                5: lambda: project_v(2),
                8: lambda: project_k(3),
                11: lambda: project_v(3),
                14: lambda: qproj(1),
            })
            normalize(0, *avs[0])
            # phases 1-3: interleave previous phase's output projection
            for p in range(1, NQP):
                pc = p - 1  # chunks 4*pc .. 4*pc+3
                fillers = {
                    4: lambda c=4 * pc + 0: outproj_chunk(c),
                    7: lambda c=4 * pc + 1: outproj_chunk(c),
                    12: lambda c=4 * pc + 2: outproj_chunk(c),
                    14: lambda c=4 * pc + 3: outproj_chunk(c),
                }
                if p + 1 < NQP:
                    fillers[9] = lambda c=p + 1: qproj(c)
                avs[p] = attention_phase(p, fillers)
                normalize(p, *avs[p])
            # tail: last phase's output projection (ACT is idle now)
            for c in range(4 * (NQP - 1), 4 * NQP):
                outproj_chunk(c, evac_scalar=True)

            if dbg:
                nc.gpsimd.dma_start(out=qt_d[:], in_=qT)
                nc.gpsimd.dma_start(out=ot_d[:], in_=oT)
                for lb in range(NLB):
                    nc.gpsimd.dma_start(
                        out=kt_d[:, lb * LB:(lb + 1) * LB], in_=ktb[lb]
                    )
                    nc.gpsimd.dma_start(
                        out=vsb_d[:, lb * 144:(lb + 1) * 144], in_=vsb[lb]
                    )

    nc.finalize()
    return nc


def _get_nc():
    if "nc" not in _CACHE:
        _CACHE["nc"] = _build_nc()
    return _CACHE["nc"]


def _pad48(w32):
    # [*, 32] -> [*, 48] with head0 dims at 0-15, head1 at 32-47
    out = np.zeros(w32.shape[:-1] + (DP,), w32.dtype)
    out[..., 0:16] = w32[..., 0:16]
    out[..., 32:48] = w32[..., 16:32]
    return out


def make_in_maps(queries, keys, values, Wq, bq, Wk, bk, Wv, bv, Wo, bo):
    f16 = np.float16
    xqt = [np.ascontiguousarray(queries[b].T).astype(f16) for b in range(B)]
    xkt = [np.ascontiguousarray(keys[b].T).astype(f16) for b in range(B)]
    xvt = [np.ascontiguousarray(values[b].T).astype(f16) for b in range(B)]
    orows = np.zeros((16, L), f16)
    orows[0] = 1.0

    in_maps = []
    for core in range(8):
        b, hg = core // 4, core % 4
        s = DSL * hg
        wo48 = np.zeros((DP, D_MODEL), np.float32)
        wo48[0:16] = Wo[s:s + 16]
        wo48[32:48] = Wo[s + 16:s + 32]
        if hg == 0:
            wo48[16] = bo
        in_maps.append({
            "xq_t": xqt[b],
            "xk_t": xkt[b],
            "xv_t": xvt[b],
            "wq": _pad48(Wq[:, s:s + DSL]).astype(f16),
            "wk": _pad48(Wk[:, s:s + DSL]).astype(f16),
            "wv": Wv[:, s:s + DSL].astype(f16),
            "bqkv": np.ascontiguousarray(
                _pad48(np.stack([bq[s:s + DSL], bk[s:s + DSL], bv[s:s + DSL]])).T
            ).astype(np.float32),
            "wo": wo48.astype(f16),
            "identp": np.eye(32, dtype=f16),
            "orows": orows,
        })
    return in_maps


def kernel(queries, keys, values, Wq, bq, Wk, bk, Wv, bv, Wo, bo, **_unused):
    queries = np.asarray(queries, dtype=np.float32)
    keys = np.asarray(keys, dtype=np.float32)
    values = np.asarray(values, dtype=np.float32)
    Wq, Wk, Wv = (np.asarray(a, dtype=np.float32) for a in (Wq, Wk, Wv))
    Wo = np.asarray(Wo, dtype=np.float32)
    bq, bk, bv, bo = (np.asarray(a, dtype=np.float32) for a in (bq, bk, bv, bo))

    nc = _get_nc()
    in_maps = make_in_maps(queries, keys, values, Wq, bq, Wk, bk, Wv, bv, Wo, bo)
    res = run_bass_kernel_spmd(nc, in_maps, core_ids=list(range(8)))
    out = np.zeros((B, L, D_MODEL), np.float32)
    for core in range(8):
        out[core // 4] += res.results[core]["y"].astype(np.float32)
    return out
